# revision 1
# baseline (speedup 1.0000x reference)
"""Multi-head attention (B=2, L=2048, D=1024, H=16, DK=64) on 8 TRN2 NeuronCores.

Sharding: core c handles batch b = c//4 and head-group g = c%4 (4 heads,
256 model dims). Per-core compute (no collectives):
  QT/KT  [256, 2048] projections in [dk, seq] layout (rhs = x^T, lhsT = w^T)
  V      [2048, 256] with a fused ones-column per head (gives softmax Z free)
  S^T    = K_h @ Q_h^T per head in [keys, queries] layout (row-packed head
           pairs on the PE array, K=64 each)
  P      = exp(S^T / 8)     one wide ACT op per (key-tile, head-pair)
  ctx^T  = V'_h^T @ P       -> rows 0:64 ctx, row 64 = Z
  ctx   /= Z                (batched reciprocal + gpsimd partition broadcast)
  outT   = w_o-block-stationary matmul -> PARTIAL output [1024, 2048] bf16
Host sums the 4 head-group partials per batch, transposes, stacks batches.

Schedule: PE warm-up matmuls cover the initial DMA latency (HAM clock
ramp); the first real matmuls need only wk + x chunk 0, which are
prioritized on both HWDGE rings.  The attention rounds run 2 key tiles
per super-iteration (longer same-stationary matmul runs, fewer weight
-load exposures); KT chunks 1-3, all V tiles, remaining Q/K projections
and the output-projection units fill the interleave slots.  The last
query chunk's hp0 output half goes to a separate DRAM partial summed on
the host, so the tail is just division -> 8 matmuls -> stores.
"""

import numpy as np

D = 1024
L = 2048
DK = 64
GH = 4           # heads per core
GD = GH * DK     # model dims per core (256)
NCORES = 8

MM_DTYPE = "bfloat16"   # matmul operand dtype: bfloat16 | float32r | float32


def _build(mm_dtype: str = MM_DTYPE):
    import os
    import concourse.bacc as bacc
    import concourse.mybir as mybir
    import concourse.tile as tile

    del os

    f32 = mybir.dt.float32
    mmdt = getattr(mybir.dt, mm_dtype)
    Exp = mybir.ActivationFunctionType.Exp

    ddt = mmdt if mm_dtype == "bfloat16" else f32
    odt = mmdt if mm_dtype == "bfloat16" else f32

    nc = bacc.Bacc("TRN2", target_bir_lowering=False, debug=False,
                   num_devices=NCORES)
    # Inputs are host-packed into the exact SBUF layouts (partition-major,
    # chunk-major for x) so every DMA line is 2-8KB contiguous -- ~3x the
    # descriptor-limited rate of the naive transposed layout.
    xh = nc.dram_tensor("xh", [128, 4, 8, 512], ddt, kind="ExternalInput").ap()
    wqh = nc.dram_tensor("wqh", [128, 8, GD], ddt, kind="ExternalInput").ap()
    wkh = nc.dram_tensor("wkh", [128, 8, GD], ddt, kind="ExternalInput").ap()
    wvh = nc.dram_tensor("wvh", [128, 8, GD], ddt, kind="ExternalInput").ap()
    woT = nc.dram_tensor("woT", [GD, D], ddt, kind="ExternalInput").ap()
    outT = nc.dram_tensor("outT", [D, L], odt, kind="ExternalOutput").ap()
    # hp0 half of the last query chunk's output partial (host adds it)
    outTb = nc.dram_tensor("outTb", [D, 512], odt, kind="ExternalOutput").ap()

    mdt = mmdt

    def dr(t):
        return t.bitcast(mmdt) if mm_dtype == "float32r" else t

    ND = D // 128    # 8 d-tiles
    NL = L // 128    # 16 key tiles
    NQ = L // 512    # 4 query chunks
    NE = D // 128    # 8 output-column blocks

    with tile.TileContext(nc) as tc:
        with (
            tc.tile_pool(name="xp", bufs=1) as xp,
            tc.tile_pool(name="wp", bufs=1) as wp,
            tc.tile_pool(name="qk", bufs=1) as qk,
            tc.tile_pool(name="vp", bufs=1) as vp,
            tc.tile_pool(name="cx", bufs=1) as cx,
            tc.tile_pool(name="pp", bufs=8) as pp,
            tc.tile_pool(name="rp", bufs=4) as rp,
            tc.tile_pool(name="op", bufs=3) as op_,
            tc.tile_pool(name="ps", bufs=2, space="PSUM") as ps,
            tc.tile_pool(name="pc", bufs=2, space="PSUM") as pc,
            tc.tile_pool(name="po", bufs=2, space="PSUM") as pop,
        ):
            # ---- PE warm-up: keep the HAM clock gate open while input
            # DMAs land.  Matmuls on a memset tile, results discarded.
            warm = wp.tile([128, 512], mdt, tag="warm", name="warm")
            nc.vector.memset(warm[:], 0.0)
            for _ in range(32):
                acc = pop.tile([128, 512], f32, tag="o", name="o")
                nc.tensor.matmul(acc[:], warm[:, 0:128], warm[:],
                                 start=True, stop=True)

            # ---- DMA inputs -------------------------------------------------
            # Critical path first on both HWDGE rings: wk + x chunk0 + wq.
            wqts = wp.tile([128, ND, GD], mdt, tag="wq", name="wq")
            wkts = wp.tile([128, ND, GD], mdt, tag="wk", name="wk")
            wvts = wp.tile([128, ND, GD], mdt, tag="wv", name="wv")
            wots = [wp.tile([128, D], mdt, tag=f"wo{i}", name=f"wo{i}")
                    for i in range(2)]
            xta = xp.tile([128, ND, L], mdt, tag="x", name="x")
            xts = [xta[:, d, :] for d in range(ND)]

            nc.sync.dma_start(wkts[:, 0:4, :], dr(wkh[:, 0:4, :]))
            nc.scalar.dma_start(wkts[:, 4:8, :], dr(wkh[:, 4:8, :]))
            nc.sync.dma_start(xta[:, 0:4, 0:512], dr(xh[:, 0, 0:4, :]))
            nc.scalar.dma_start(xta[:, 4:8, 0:512], dr(xh[:, 0, 4:8, :]))
            nc.sync.dma_start(wqts[:, 0:4, :], dr(wqh[:, 0:4, :]))
            nc.scalar.dma_start(wqts[:, 4:8, :], dr(wqh[:, 4:8, :]))
            for qtr in range(1, 4):
                sl = slice(qtr * 512, (qtr + 1) * 512)
                nc.sync.dma_start(xta[:, 0:4, sl], dr(xh[:, qtr, 0:4, :]))
                nc.scalar.dma_start(xta[:, 4:8, sl], dr(xh[:, qtr, 4:8, :]))
            for i in range(2):
                nc.gpsimd.dma_start(wvts[:, 4 * i:4 * i + 4, :],
                                    dr(wvh[:, 4 * i:4 * i + 4, :]))
            for i in range(2):
                nc.gpsimd.dma_start(wots[i][:], dr(woT[i * 128:(i + 1) * 128, :]))

            # ---- projection helpers (emitted just-in-time) ----------------
            vpa = vp.tile([128, NL * GH, DK + 1], mdt, tag="v", name="v")
            nc.vector.memset(vpa[:, :, DK:DK + 1], 1.0)

            qth = [qk.tile([128, L], mdt, tag=f"q{hp}", name=f"q{hp}")
                   for hp in range(2)]
            kth = [qk.tile([128, L], mdt, tag=f"k{hp}", name=f"k{hp}")
                   for hp in range(2)]
            ctxt = [cx.tile([128, L], mdt, tag=f"c{hp}", name=f"c{hp}")
                    for hp in range(2)]

            def proj_qk_chunk(hp, w_sb, dstl, qc):
                acc = pop.tile([128, 512], f32, tag="o", name="o")
                for d in range(ND):
                    nc.tensor.matmul(
                        acc[:], w_sb[:, d, hp * 128:(hp + 1) * 128],
                        xts[d][:, qc * 512:(qc + 1) * 512],
                        start=(d == 0), stop=(d == ND - 1))
                nc.vector.tensor_copy(
                    dstl[hp][:, qc * 512:(qc + 1) * 512], acc[:])

            def v_proj_tile(lt):
                acc = pop.tile([128, 512], f32, tag="o", name="o")
                for d in range(ND):
                    nc.tensor.matmul(
                        acc[:, 0:GD],
                        xts[d][:, lt * 128:(lt + 1) * 128],
                        wvts[:, d, :],
                        start=(d == 0), stop=(d == ND - 1))
                nc.vector.tensor_copy(
                    vpa[:, lt * GH:(lt + 1) * GH, 0:DK],
                    acc[:, 0:GD].rearrange("p (h c) -> p h c", h=GH))

            def out_unit(eb, qc):
                esl = slice(eb * 128, (eb + 1) * 128)
                qsl = slice(qc * 512, (qc + 1) * 512)
                po = pop.tile([128, 512], f32, tag="o", name="o")
                for hp in range(2):
                    nc.tensor.matmul(po[:], wots[hp][:, esl],
                                     ctxt[hp][:, qsl],
                                     start=(hp == 0), stop=(hp == 1))
                ot = op_.tile([128, 512], mdt, tag="ot", name="ot")
                nc.vector.tensor_copy(ot[:], po[:])
                nc.sync.dma_start(outT[esl, qsl], ot[:])

            def oas_unit(eb):
                # hp0 half of the last query chunk -> separate DRAM partial
                qsl = slice((NQ - 1) * 512, NQ * 512)
                esl = slice(eb * 128, (eb + 1) * 128)
                po = pop.tile([128, 512], f32, tag="o", name="o")
                nc.tensor.matmul(po[:], wots[0][:, esl], ctxt[0][:, qsl],
                                 start=True, stop=True)
                ot = op_.tile([128, 512], mdt, tag="ot", name="ot")
                nc.vector.tensor_copy(ot[:], po[:])
                nc.scalar.dma_start(outTb[esl, :], ot[:])

            def attn(hp, qc, interleave=None, fast_div=False):
                """Attention for head-pair hp, query chunk qc.

                Two key tiles per super-iteration: scores for both tiles
                back-to-back (4 matmuls), then both exps, then both ctx
                accumulations (4 matmuls) -- longer same-kind runs hide
                more weight loads.  interleave: dict lt2 -> callable run
                between the scores and the ctx block of that super-iter.
                """
                qsl = slice(qc * 512, (qc + 1) * 512)
                cps = [pc.tile([DK + 1, 512], f32, tag="c", name=f"c{i}")
                       for i in range(2)]
                for lt2 in range(0, NL, 2):
                    sps = []
                    for lt in (lt2, lt2 + 1):
                        lsl = slice(lt * 128, (lt + 1) * 128)
                        sp = ps.tile([128, 1024], f32, tag="s", name="s")
                        sps.append(sp)
                        for i in range(2):
                            row = slice(i * 64, (i + 1) * 64)
                            nc.tensor.matmul(
                                sp[:, i * 512:(i + 1) * 512],
                                kth[hp][row, lsl], qth[hp][row, qsl],
                                start=True, stop=True,
                                tile_position=(i * 64, 0))
                    if interleave is not None and lt2 in interleave:
                        interleave[lt2]()
                    pts = []
                    for j, lt in enumerate((lt2, lt2 + 1)):
                        p = pp.tile([128, 1024], mdt, tag="p", name="p")
                        pts.append(p)
                        nc.scalar.activation(p[:], sps[j][:], Exp, scale=0.125)
                    for j, lt in enumerate((lt2, lt2 + 1)):
                        for i in range(2):
                            nc.tensor.matmul(
                                cps[i][:], vpa[:, lt * GH + 2 * hp + i, :],
                                pts[j][:, i * 512:(i + 1) * 512],
                                start=(lt == 0), stop=(lt == NL - 1))
                # divide by Z: evacuate both heads' psum, then reciprocals,
                # then broadcasts+muls.  fast_div orders for latency (the
                # reciprocals read the Z rows straight from PSUM so the
                # gpsimd broadcasts launch ASAP, overlapping the ctx
                # evacuation casts) -- used for the final round, whose
                # division gates the output tail.
                if fast_div:
                    # latency order: Z rows out first, reciprocals and
                    # gpsimd broadcasts launch while the big ctx casts run
                    zis = []
                    for i in range(2):
                        zi = rp.tile([1, 512], f32, tag="zi", name="zi")
                        nc.vector.tensor_copy(zi[:], cps[i][DK:DK + 1, :])
                        zis.append(zi)
                    rzs = []
                    for i in range(2):
                        rz = rp.tile([1, 512], f32, tag="rz", name="rz")
                        nc.vector.reciprocal_approx_fast(rz[:], zis[i][:])
                        rzs.append(rz)
                    rzbs = []
                    for i in range(2):
                        rzb = rp.tile([64, 512], f32, tag="rzb", name="rzb")
                        nc.gpsimd.partition_broadcast(rzb[:], rzs[i][:])
                        rzbs.append(rzb)
                    cus = []
                    for i in range(2):
                        cu = pp.tile([64, 512], mdt, tag=f"cu{2*hp+i}",
                                     name=f"cu{2*hp+i}", bufs=2)
                        nc.vector.tensor_copy(cu[:], cps[i][0:DK, :])
                        cus.append(cu)
                    for i in range(2):
                        nc.vector.tensor_mul(
                            ctxt[hp][i * 64:(i + 1) * 64, qsl],
                            cus[i][:], rzbs[i][:])
                    return
                cus, rzs = [], []
                for i in range(2):
                    cu = pp.tile([64, 512], mdt, tag=f"cu{2*hp+i}",
                                 name=f"cu{2*hp+i}", bufs=2)
                    nc.vector.tensor_copy(cu[:], cps[i][0:DK, :])
                    zi = rp.tile([1, 512], f32, tag="zi", name="zi")
                    nc.vector.tensor_copy(zi[:], cps[i][DK:DK + 1, :])
                    cus.append(cu)
                    rzs.append(zi)
                for i in range(2):
                    rz = rp.tile([1, 512], f32, tag="rz", name="rz")
                    nc.vector.reciprocal_approx_fast(rz[:], rzs[i][:])
                    rzs[i] = rz
                for i in range(2):
                    rzb = rp.tile([64, 512], f32, tag="rzb", name="rzb")
                    nc.gpsimd.partition_broadcast(rzb[:], rzs[i][:])
                    nc.vector.tensor_mul(
                        ctxt[hp][i * 64:(i + 1) * 64, qsl],
                        cus[i][:], rzb[:])

            # ---- schedule ---------------------------------------------------
            proj_qk_chunk(0, wkts, kth, 0)
            proj_qk_chunk(0, wqts, qth, 0)

            def u(fn, *a):
                return lambda: fn(*a)

            def merge(*fns):
                def f():
                    for g in fns:
                        g()
                return f

            def vpp(a, b, *extra):
                return merge(u(v_proj_tile, a), u(v_proj_tile, b), *extra)

            # Round (0,0): all V tiles (needed by its own ctx), KT0 chunk
            # c needed by the scores of super-iter 4c, KT1 + QT1 c0 for
            # the next round.  High PE density is essential: the chip
            # throttles clocks when the PE stream goes sparse.
            attn(0, 0, interleave={
                0: vpp(0, 1),
                2: vpp(2, 3, u(proj_qk_chunk, 0, wkts, kth, 1)),
                4: vpp(4, 5, u(proj_qk_chunk, 1, wkts, kth, 0)),
                6: vpp(6, 7, u(proj_qk_chunk, 0, wkts, kth, 2)),
                8: vpp(8, 9, u(proj_qk_chunk, 1, wqts, qth, 0)),
                10: vpp(10, 11, u(proj_qk_chunk, 0, wkts, kth, 3)),
                12: vpp(12, 13, u(proj_qk_chunk, 1, wkts, kth, 1)),
                14: vpp(14, 15),
            })
            attn(1, 0, interleave={
                0: u(proj_qk_chunk, 1, wkts, kth, 2),
                4: u(proj_qk_chunk, 1, wkts, kth, 3),
                6: u(proj_qk_chunk, 0, wqts, qth, 1),
            })
            # out units lag their query chunk by one round and start at
            # slot 4 (the producing division trails the previous round).
            attn(0, 1, interleave={
                0: u(proj_qk_chunk, 1, wqts, qth, 1),
                4: u(out_unit, 0, 0), 8: u(out_unit, 1, 0),
                12: u(out_unit, 2, 0), 14: u(out_unit, 3, 0),
            })
            attn(1, 1, interleave={
                0: u(proj_qk_chunk, 0, wqts, qth, 2),
                4: u(out_unit, 4, 0), 8: u(out_unit, 5, 0),
                12: u(out_unit, 6, 0), 14: u(out_unit, 7, 0),
            })
            attn(0, 2, interleave={
                0: u(proj_qk_chunk, 1, wqts, qth, 2),
                4: u(out_unit, 0, 1), 8: u(out_unit, 1, 1),
                12: u(out_unit, 2, 1), 14: u(out_unit, 3, 1),
            })
            attn(1, 2, interleave={
                0: u(proj_qk_chunk, 0, wqts, qth, 3),
                4: u(out_unit, 4, 1), 8: u(out_unit, 5, 1),
                12: u(out_unit, 6, 1), 14: u(out_unit, 7, 1),
            })
            attn(0, 3, interleave={
                0: u(proj_qk_chunk, 1, wqts, qth, 3),
                4: u(out_unit, 0, 2), 8: u(out_unit, 1, 2),
                12: u(out_unit, 2, 2), 14: u(out_unit, 3, 2),
            })
            attn(1, 3, interleave={
                2: merge(u(out_unit, 4, 2), u(out_unit, 5, 2)),
                4: merge(u(out_unit, 6, 2), u(out_unit, 7, 2)),
                6: merge(u(oas_unit, 0), u(oas_unit, 1)),
                8: merge(u(oas_unit, 2), u(oas_unit, 3)),
                10: merge(u(oas_unit, 4), u(oas_unit, 5)),
                12: merge(u(oas_unit, 6), u(oas_unit, 7)),
            }, fast_div=True)
            # Tail: hp1 halves of the last chunk.  Evacuations alternate
            # between the vector and scalar engines and the stores between
            # both HWDGE rings so the drain pipelines.
            qsl = slice((NQ - 1) * 512, NQ * 512)
            for eb in range(NE):
                esl = slice(eb * 128, (eb + 1) * 128)
                po = pop.tile([128, 512], f32, tag="o", name="o")
                nc.tensor.matmul(po[:], wots[1][:, esl], ctxt[1][:, qsl],
                                 start=True, stop=True)
                ot = op_.tile([128, 512], mdt, tag="ot", name="ot")
                nc.vector.tensor_copy(ot[:], po[:])
                if eb % 2 == 0:
                    nc.sync.dma_start(outT[esl, qsl], ot[:])
                else:
                    nc.scalar.dma_start(outT[esl, qsl], ot[:])
    nc.compile()
    return nc


_CACHED = {}


def _get_nc(mm_dtype: str = MM_DTYPE):
    if mm_dtype not in _CACHED:
        _CACHED[mm_dtype] = _build(mm_dtype)
    return _CACHED[mm_dtype]


def _round_fp32r(a):
    """Round-to-nearest-even fp32 -> fp32r (11 explicit mantissa bits)."""
    u = np.ascontiguousarray(a, np.float32).view(np.uint32).copy()
    u += 0x7FF + ((u >> 12) & 1)
    u &= 0xFFFFF000
    return u.view(np.float32)


def make_in_maps(x, w_qkv, w_o):
    if MM_DTYPE == "float32r":
        cvt = _round_fp32r
    elif MM_DTYPE == "bfloat16":
        import ml_dtypes
        cvt = lambda a: np.asarray(a, dtype=ml_dtypes.bfloat16)  # noqa: E731
    else:
        cvt = lambda a: a  # noqa: E731
    wq, wk, wv = (w_qkv[i * D:(i + 1) * D] for i in range(3))

    def pack_x(xb):
        # [L, D] -> [128, NQ, ND, 512]: xh[p, qc, d, c] = xb[qc*512+c, d*128+p]
        return np.ascontiguousarray(
            xb.reshape(4, 512, 8, 128).transpose(3, 0, 2, 1))

    def pack_w(wg):
        # [GD, D] -> [128, ND, GD]: wh[p, d, c] = wg.T[d*128+p, c]
        return np.ascontiguousarray(
            wg.T.reshape(8, 128, GD).transpose(1, 0, 2))

    in_maps = []
    for c in range(NCORES):
        b, g = divmod(c, 4)
        gs = slice(g * GD, (g + 1) * GD)
        in_maps.append({
            "xh": cvt(pack_x(x[b])),
            "wqh": cvt(pack_w(wq[gs])),
            "wkh": cvt(pack_w(wk[gs])),
            "wvh": cvt(pack_w(wv[gs])),
            "woT": cvt(np.ascontiguousarray(w_o[:, gs].T)),
        })
    return in_maps


def assemble(results):
    out = np.empty((2, L, D), np.float32)
    for b in range(2):
        acc = np.asarray(results[4 * b]["outT"], np.float32)
        accb = np.asarray(results[4 * b]["outTb"], np.float32)
        for g in range(1, 4):
            acc = acc + np.asarray(results[4 * b + g]["outT"], np.float32)
            accb = accb + np.asarray(results[4 * b + g]["outTb"], np.float32)
        acc[:, (L - 512):] += accb
        out[b] = acc.T
    return out


def kernel(x, w_qkv, w_o):
    from concourse import bass_utils
    nc = _get_nc()
    in_maps = make_in_maps(np.asarray(x, np.float32),
                           np.asarray(w_qkv, np.float32),
                           np.asarray(w_o, np.float32))
    res = bass_utils.run_bass_kernel_spmd(
        nc, in_maps, core_ids=list(range(NCORES)))
    return assemble(res.results)



# revision 11
# speedup vs baseline: 1.0060x; 1.0060x over previous
"""Multi-head attention (B=2, L=2048, D=1024, H=16, DK=64) on 8 TRN2 NeuronCores.

Sharding: core c handles batch b = c//4 and head-group g = c%4 (4 heads,
256 model dims). Per-core compute (no collectives):
  QT/KT  [256, 2048] projections in [dk, seq] layout (rhs = x^T, lhsT = w^T)
  V      [2048, 256] with a fused ones-column per head (gives softmax Z free)
  S^T    = K_h @ Q_h^T per head in [keys, queries] layout (row-packed head
           pairs on the PE array, K=64 each)
  P      = exp(S^T / 8)     one wide ACT op per (key-tile, head-pair)
  ctx^T  = V'_h^T @ P       -> rows 0:64 ctx, row 64 = Z
  ctx   /= Z                (batched reciprocal + gpsimd partition broadcast)
  outT   = w_o-block-stationary matmul -> PARTIAL output [1024, 2048] bf16
Host sums the 4 head-group partials per batch, transposes, stacks batches.

Schedule: PE warm-up matmuls cover the initial DMA latency (HAM clock
ramp); the first real matmuls need only wk + x chunk 0, which are
prioritized on both HWDGE rings.  The attention rounds run 2 key tiles
per super-iteration (longer same-stationary matmul runs, fewer weight
-load exposures); KT chunks 1-3, all V tiles, remaining Q/K projections
and the output-projection units fill the interleave slots.  The last
query chunk's hp0 output half goes to a separate DRAM partial summed on
the host, so the tail is just division -> 8 matmuls -> stores.
"""

import numpy as np

D = 1024
L = 2048
DK = 64
GH = 4           # heads per core
GD = GH * DK     # model dims per core (256)
NCORES = 8

MM_DTYPE = "bfloat16"   # matmul operand dtype: bfloat16 | float32r | float32


def _build(mm_dtype: str = MM_DTYPE):
    import os
    import concourse.bacc as bacc
    import concourse.mybir as mybir
    import concourse.tile as tile

    del os

    f32 = mybir.dt.float32
    mmdt = getattr(mybir.dt, mm_dtype)
    Exp = mybir.ActivationFunctionType.Exp
    Copy = mybir.ActivationFunctionType.Copy

    ddt = mmdt if mm_dtype == "bfloat16" else f32
    odt = mmdt if mm_dtype == "bfloat16" else f32

    nc = bacc.Bacc("TRN2", target_bir_lowering=False, debug=False,
                   num_devices=NCORES)
    # Inputs are host-packed into the exact SBUF layouts (partition-major,
    # chunk-major for x) so every DMA line is 2-8KB contiguous -- ~3x the
    # descriptor-limited rate of the naive transposed layout.
    xh = nc.dram_tensor("xh", [128, 4, 8, 512], ddt, kind="ExternalInput").ap()
    wqh = nc.dram_tensor("wqh", [128, 8, GD], ddt, kind="ExternalInput").ap()
    wkh = nc.dram_tensor("wkh", [128, 8, GD], ddt, kind="ExternalInput").ap()
    wvh = nc.dram_tensor("wvh", [128, 8, GD], ddt, kind="ExternalInput").ap()
    woT = nc.dram_tensor("woT", [GD, D], ddt, kind="ExternalInput").ap()
    outT = nc.dram_tensor("outT", [D, L], odt, kind="ExternalOutput").ap()
    # hp0 half of the last query chunk's output partial (host adds it)
    outTb = nc.dram_tensor("outTb", [D, 512], odt, kind="ExternalOutput").ap()

    mdt = mmdt

    def dr(t):
        return t.bitcast(mmdt) if mm_dtype == "float32r" else t

    ND = D // 128    # 8 d-tiles
    NL = L // 128    # 16 key tiles
    NQ = L // 512    # 4 query chunks
    NE = D // 128    # 8 output-column blocks

    with tile.TileContext(nc) as tc:
        with (
            tc.tile_pool(name="xp", bufs=1) as xp,
            tc.tile_pool(name="wp", bufs=1) as wp,
            tc.tile_pool(name="qk", bufs=1) as qk,
            tc.tile_pool(name="vp", bufs=1) as vp,
            tc.tile_pool(name="cx", bufs=1) as cx,
            tc.tile_pool(name="pp", bufs=8) as pp,
            tc.tile_pool(name="rp", bufs=4) as rp,
            tc.tile_pool(name="op", bufs=3) as op_,
            tc.tile_pool(name="ps", bufs=2, space="PSUM") as ps,
            tc.tile_pool(name="pc", bufs=2, space="PSUM") as pc,
            tc.tile_pool(name="po", bufs=2, space="PSUM") as pop,
        ):
            # ---- PE warm-up: keep the HAM clock gate open while input
            # DMAs land.  Matmuls on a memset tile, results discarded.
            # Short: the first real projections are DMA-paced anyway.
            warm = wp.tile([128, 512], mdt, tag="warm", name="warm")
            nc.vector.memset(warm[:], 0.0)
            for _ in range(6):
                acc = pop.tile([128, 512], f32, tag="o", name="o")
                nc.tensor.matmul(acc[:], warm[:, 0:128], warm[:],
                                 start=True, stop=True)

            # ---- DMA inputs -------------------------------------------------
            # Critical path first on both HWDGE rings, split per d-tile so
            # the first projection's matmuls fire as each (wk_d, x_d) pair
            # lands instead of waiting for the whole block: sync ring carries
            # d 0-3, scalar ring d 4-7, pair-interleaved.
            wqts = wp.tile([128, ND, GD], mdt, tag="wq", name="wq")
            wkts = wp.tile([128, ND, GD], mdt, tag="wk", name="wk")
            wvts = wp.tile([128, ND, GD], mdt, tag="wv", name="wv")
            wots = [wp.tile([128, D], mdt, tag=f"wo{i}", name=f"wo{i}")
                    for i in range(2)]
            xta = xp.tile([128, ND, L], mdt, tag="x", name="x")
            xts = [xta[:, d, :] for d in range(ND)]

            for j in range(4):
                nc.sync.dma_start(wkts[:, j, :], dr(wkh[:, j, :]))
                nc.sync.dma_start(xta[:, j, 0:512], dr(xh[:, 0, j, :]))
                nc.scalar.dma_start(wkts[:, 4 + j, :], dr(wkh[:, 4 + j, :]))
                nc.scalar.dma_start(xta[:, 4 + j, 0:512],
                                    dr(xh[:, 0, 4 + j, :]))
            nc.sync.dma_start(wqts[:, 0:4, :], dr(wqh[:, 0:4, :]))
            nc.scalar.dma_start(wqts[:, 4:8, :], dr(wqh[:, 4:8, :]))
            for qtr in range(1, 4):
                sl = slice(qtr * 512, (qtr + 1) * 512)
                nc.sync.dma_start(xta[:, 0:4, sl], dr(xh[:, qtr, 0:4, :]))
                nc.scalar.dma_start(xta[:, 4:8, sl], dr(xh[:, qtr, 4:8, :]))
            for i in range(2):
                nc.gpsimd.dma_start(wvts[:, 4 * i:4 * i + 4, :],
                                    dr(wvh[:, 4 * i:4 * i + 4, :]))
            for i in range(2):
                nc.gpsimd.dma_start(wots[i][:], dr(woT[i * 128:(i + 1) * 128, :]))

            # All-ones row for the final division: a K=1 matmul with this
            # stationary replicates a [1, 512] row across 64 PSUM partitions
            # (PE-based partition broadcast of 1/Z).
            bc_pat = wp.tile([1, 64], f32, tag="bcp", name="bcp")
            nc.vector.memset(bc_pat[:], 1.0)

            # ---- projection helpers (emitted just-in-time) ----------------
            vpa = vp.tile([128, NL * GH, DK + 1], mdt, tag="v", name="v")
            nc.vector.memset(vpa[:, :, DK:DK + 1], 1.0)

            qth = [qk.tile([128, L], mdt, tag=f"q{hp}", name=f"q{hp}")
                   for hp in range(2)]
            kth = [qk.tile([128, L], mdt, tag=f"k{hp}", name=f"k{hp}")
                   for hp in range(2)]
            ctxt = [cx.tile([128, L], mdt, tag=f"c{hp}", name=f"c{hp}")
                    for hp in range(2)]

            def proj_qk_chunk(hp, w_sb, dstl, qc, dorder=None):
                acc = pop.tile([128, 512], f32, tag="o", name="o")
                order = list(dorder) if dorder is not None else list(range(ND))
                for i, d in enumerate(order):
                    nc.tensor.matmul(
                        acc[:], w_sb[:, d, hp * 128:(hp + 1) * 128],
                        xts[d][:, qc * 512:(qc + 1) * 512],
                        start=(i == 0), stop=(i == ND - 1))
                nc.vector.tensor_copy(
                    dstl[hp][:, qc * 512:(qc + 1) * 512], acc[:])

            def v_proj_tile(lt):
                acc = pop.tile([128, 512], f32, tag="o", name="o")
                for d in range(ND):
                    nc.tensor.matmul(
                        acc[:, 0:GD],
                        xts[d][:, lt * 128:(lt + 1) * 128],
                        wvts[:, d, :],
                        start=(d == 0), stop=(d == ND - 1))
                nc.vector.tensor_copy(
                    vpa[:, lt * GH:(lt + 1) * GH, 0:DK],
                    acc[:, 0:GD].rearrange("p (h c) -> p h c", h=GH))

            def out_unit(eb, qc):
                esl = slice(eb * 128, (eb + 1) * 128)
                qsl = slice(qc * 512, (qc + 1) * 512)
                po = pop.tile([128, 512], f32, tag="o", name="o")
                for hp in range(2):
                    nc.tensor.matmul(po[:], wots[hp][:, esl],
                                     ctxt[hp][:, qsl],
                                     start=(hp == 0), stop=(hp == 1))
                ot = op_.tile([128, 512], mdt, tag="ot", name="ot")
                nc.vector.tensor_copy(ot[:], po[:])
                nc.sync.dma_start(outT[esl, qsl], ot[:])

            def oas_unit(eb):
                # hp0 half of the last query chunk -> separate DRAM partial
                qsl = slice((NQ - 1) * 512, NQ * 512)
                esl = slice(eb * 128, (eb + 1) * 128)
                po = pop.tile([128, 512], f32, tag="o", name="o")
                nc.tensor.matmul(po[:], wots[0][:, esl], ctxt[0][:, qsl],
                                 start=True, stop=True)
                ot = op_.tile([128, 512], mdt, tag="ot", name="ot")
                nc.vector.tensor_copy(ot[:], po[:])
                nc.scalar.dma_start(outTb[esl, :], ot[:])

            def attn(hp, qc, interleave=None, fast_div=False):
                """Attention for head-pair hp, query chunk qc.

                Two key tiles per super-iteration: scores for both tiles
                back-to-back (4 matmuls), then both exps, then both ctx
                accumulations (4 matmuls) -- longer same-kind runs hide
                more weight loads.  interleave: dict lt2 -> callable run
                between the scores and the ctx block of that super-iter.
                """
                qsl = slice(qc * 512, (qc + 1) * 512)
                cps = [pc.tile([DK + 1, 512], f32, tag="c", name=f"c{i}")
                       for i in range(2)]
                for lt2 in range(0, NL, 2):
                    sps = []
                    for lt in (lt2, lt2 + 1):
                        lsl = slice(lt * 128, (lt + 1) * 128)
                        sp = ps.tile([128, 1024], f32, tag="s", name="s")
                        sps.append(sp)
                        for i in range(2):
                            row = slice(i * 64, (i + 1) * 64)
                            nc.tensor.matmul(
                                sp[:, i * 512:(i + 1) * 512],
                                kth[hp][row, lsl], qth[hp][row, qsl],
                                start=True, stop=True,
                                tile_position=(i * 64, 0))
                    if interleave is not None and lt2 in interleave:
                        interleave[lt2]()
                    pts = []
                    for j, lt in enumerate((lt2, lt2 + 1)):
                        p = pp.tile([128, 1024], mdt, tag="p", name="p")
                        pts.append(p)
                        nc.scalar.activation(p[:], sps[j][:], Exp, scale=0.125)
                    for j, lt in enumerate((lt2, lt2 + 1)):
                        for i in range(2):
                            nc.tensor.matmul(
                                cps[i][:], vpa[:, lt * GH + 2 * hp + i, :],
                                pts[j][:, i * 512:(i + 1) * 512],
                                start=(lt == 0), stop=(lt == NL - 1))
                # divide by Z: evacuate both heads' psum, then reciprocals,
                # then broadcasts+muls.  fast_div orders for latency (the
                # reciprocals read the Z rows straight from PSUM so the
                # gpsimd broadcasts launch ASAP, overlapping the ctx
                # evacuation casts) -- used for the final round, whose
                # division gates the output tail.
                if fast_div:
                    # latency order: reciprocals read the Z rows straight
                    # from PSUM; 1/Z is partition-broadcast by a tiny PE
                    # matmul (also keeps the PE clock warm for the tail);
                    # the raw-ctx casts run on the scalar engine in
                    # parallel; one wide multiply normalizes both heads.
                    zz = rp.tile([1, 1024], f32, tag="zz2", name="zz2")
                    nc.vector.tensor_copy(zz[0:1, 0:512],
                                          cps[0][DK:DK + 1, :])
                    nc.scalar.activation(zz[0:1, 512:1024],
                                         cps[1][DK:DK + 1, :], Copy)
                    rz = rp.tile([1, 1024], f32, tag="rz2", name="rz2")
                    nc.vector.reciprocal_approx_fast(rz[:], zz[:])
                    cu = pp.tile([128, 512], mdt, tag="cuf", name="cuf")
                    for i in range(2):
                        nc.scalar.activation(cu[i * 64:(i + 1) * 64, :],
                                             cps[i][0:DK, :], Copy)
                    rzb = pop.tile([128, 512], f32, tag="o", name="o")
                    for i in range(2):
                        nc.tensor.matmul(rzb[i * 64:(i + 1) * 64, :],
                                         bc_pat[:],
                                         rz[0:1, i * 512:(i + 1) * 512],
                                         start=True, stop=True)
                    nc.vector.tensor_mul(ctxt[hp][:, qsl], cu[:], rzb[:])
                    return
                cus, rzs = [], []
                for i in range(2):
                    cu = pp.tile([64, 512], mdt, tag=f"cu{2*hp+i}",
                                 name=f"cu{2*hp+i}", bufs=2)
                    nc.vector.tensor_copy(cu[:], cps[i][0:DK, :])
                    zi = rp.tile([1, 512], f32, tag="zi", name="zi")
                    nc.vector.tensor_copy(zi[:], cps[i][DK:DK + 1, :])
                    cus.append(cu)
                    rzs.append(zi)
                for i in range(2):
                    rz = rp.tile([1, 512], f32, tag="rz", name="rz")
                    nc.vector.reciprocal_approx_fast(rz[:], rzs[i][:])
                    rzs[i] = rz
                for i in range(2):
                    rzb = rp.tile([64, 512], f32, tag="rzb", name="rzb")
                    nc.gpsimd.partition_broadcast(rzb[:], rzs[i][:])
                    nc.vector.tensor_mul(
                        ctxt[hp][i * 64:(i + 1) * 64, qsl],
                        cus[i][:], rzb[:])

            # ---- schedule ---------------------------------------------------
            # First projections consume d-tiles in DMA landing order
            # (sync ring delivers d 0-3 while scalar delivers d 4-7).
            DORD = [0, 4, 1, 5, 2, 6, 3, 7]
            proj_qk_chunk(0, wkts, kth, 0, dorder=DORD)
            proj_qk_chunk(0, wqts, qth, 0, dorder=DORD)

            def u(fn, *a):
                return lambda: fn(*a)

            def merge(*fns):
                def f():
                    for g in fns:
                        g()
                return f

            def vpp(a, b, *extra):
                return merge(u(v_proj_tile, a), u(v_proj_tile, b), *extra)

            # Round (0,0): all V tiles (needed by its own ctx), KT0 chunk
            # c needed by the scores of super-iter 4c, KT1 + QT1 c0 for
            # the next round.  High PE density is essential: the chip
            # throttles clocks when the PE stream goes sparse.
            attn(0, 0, interleave={
                0: vpp(0, 1),
                2: vpp(2, 3, u(proj_qk_chunk, 0, wkts, kth, 1)),
                4: vpp(4, 5, u(proj_qk_chunk, 1, wkts, kth, 0)),
                6: vpp(6, 7, u(proj_qk_chunk, 0, wkts, kth, 2)),
                8: vpp(8, 9, u(proj_qk_chunk, 1, wqts, qth, 0)),
                10: vpp(10, 11, u(proj_qk_chunk, 0, wkts, kth, 3)),
                12: vpp(12, 13, u(proj_qk_chunk, 1, wkts, kth, 1)),
                14: vpp(14, 15),
            })
            attn(1, 0, interleave={
                0: u(proj_qk_chunk, 1, wkts, kth, 2),
                4: u(proj_qk_chunk, 1, wkts, kth, 3),
                6: u(proj_qk_chunk, 0, wqts, qth, 1),
            })
            # out units lag their query chunk by one round and start at
            # slot 4 (the producing division trails the previous round).
            attn(0, 1, interleave={
                0: u(proj_qk_chunk, 1, wqts, qth, 1),
                4: u(out_unit, 0, 0), 8: u(out_unit, 1, 0),
                12: u(out_unit, 2, 0), 14: u(out_unit, 3, 0),
            })
            attn(1, 1, interleave={
                0: u(proj_qk_chunk, 0, wqts, qth, 2),
                4: u(out_unit, 4, 0), 8: u(out_unit, 5, 0),
                12: u(out_unit, 6, 0), 14: u(out_unit, 7, 0),
            })
            attn(0, 2, interleave={
                0: u(proj_qk_chunk, 1, wqts, qth, 2),
                4: u(out_unit, 0, 1), 8: u(out_unit, 1, 1),
                12: u(out_unit, 2, 1), 14: u(out_unit, 3, 1),
            })
            attn(1, 2, interleave={
                0: u(proj_qk_chunk, 0, wqts, qth, 3),
                4: u(out_unit, 4, 1), 8: u(out_unit, 5, 1),
                12: u(out_unit, 6, 1), 14: u(out_unit, 7, 1),
            })
            attn(0, 3, interleave={
                0: u(proj_qk_chunk, 1, wqts, qth, 3),
                4: u(out_unit, 0, 2), 8: u(out_unit, 1, 2),
                12: u(out_unit, 2, 2), 14: u(out_unit, 3, 2),
            })
            attn(1, 3, interleave={
                2: merge(u(out_unit, 4, 2), u(out_unit, 5, 2)),
                4: merge(u(out_unit, 6, 2), u(out_unit, 7, 2)),
                6: merge(u(oas_unit, 0), u(oas_unit, 1)),
                8: merge(u(oas_unit, 2), u(oas_unit, 3)),
                10: merge(u(oas_unit, 4), u(oas_unit, 5)),
                12: merge(u(oas_unit, 6), u(oas_unit, 7)),
            }, fast_div=True)
            # Tail: hp1 halves of the last chunk.  Evacuations alternate
            # between the vector and scalar engines and the stores between
            # both HWDGE rings so the drain pipelines.
            qsl = slice((NQ - 1) * 512, NQ * 512)
            for eb in range(NE):
                esl = slice(eb * 128, (eb + 1) * 128)
                po = pop.tile([128, 512], f32, tag="o", name="o")
                nc.tensor.matmul(po[:], wots[1][:, esl], ctxt[1][:, qsl],
                                 start=True, stop=True)
                ot = op_.tile([128, 512], mdt, tag="ot", name="ot")
                if eb % 2 == 0:
                    nc.vector.tensor_copy(ot[:], po[:])
                    nc.sync.dma_start(outT[esl, qsl], ot[:])
                else:
                    nc.scalar.activation(ot[:], po[:], Copy)
                    nc.scalar.dma_start(outT[esl, qsl], ot[:])
    nc.compile()
    return nc


_CACHED = {}


def _get_nc(mm_dtype: str = MM_DTYPE):
    if mm_dtype not in _CACHED:
        _CACHED[mm_dtype] = _build(mm_dtype)
    return _CACHED[mm_dtype]


def _round_fp32r(a):
    """Round-to-nearest-even fp32 -> fp32r (11 explicit mantissa bits)."""
    u = np.ascontiguousarray(a, np.float32).view(np.uint32).copy()
    u += 0x7FF + ((u >> 12) & 1)
    u &= 0xFFFFF000
    return u.view(np.float32)


def make_in_maps(x, w_qkv, w_o):
    if MM_DTYPE == "float32r":
        cvt = _round_fp32r
    elif MM_DTYPE == "bfloat16":
        import ml_dtypes
        cvt = lambda a: np.asarray(a, dtype=ml_dtypes.bfloat16)  # noqa: E731
    else:
        cvt = lambda a: a  # noqa: E731
    wq, wk, wv = (w_qkv[i * D:(i + 1) * D] for i in range(3))

    def pack_x(xb):
        # [L, D] -> [128, NQ, ND, 512]: xh[p, qc, d, c] = xb[qc*512+c, d*128+p]
        return np.ascontiguousarray(
            xb.reshape(4, 512, 8, 128).transpose(3, 0, 2, 1))

    def pack_w(wg):
        # [GD, D] -> [128, ND, GD]: wh[p, d, c] = wg.T[d*128+p, c]
        return np.ascontiguousarray(
            wg.T.reshape(8, 128, GD).transpose(1, 0, 2))

    in_maps = []
    for c in range(NCORES):
        b, g = divmod(c, 4)
        gs = slice(g * GD, (g + 1) * GD)
        in_maps.append({
            "xh": cvt(pack_x(x[b])),
            "wqh": cvt(pack_w(wq[gs])),
            "wkh": cvt(pack_w(wk[gs])),
            "wvh": cvt(pack_w(wv[gs])),
            "woT": cvt(np.ascontiguousarray(w_o[:, gs].T)),
        })
    return in_maps


def assemble(results):
    out = np.empty((2, L, D), np.float32)
    for b in range(2):
        acc = np.asarray(results[4 * b]["outT"], np.float32)
        accb = np.asarray(results[4 * b]["outTb"], np.float32)
        for g in range(1, 4):
            acc = acc + np.asarray(results[4 * b + g]["outT"], np.float32)
            accb = accb + np.asarray(results[4 * b + g]["outTb"], np.float32)
        acc[:, (L - 512):] += accb
        out[b] = acc.T
    return out


def kernel(x, w_qkv, w_o):
    from concourse import bass_utils
    nc = _get_nc()
    in_maps = make_in_maps(np.asarray(x, np.float32),
                           np.asarray(w_qkv, np.float32),
                           np.asarray(w_o, np.float32))
    res = bass_utils.run_bass_kernel_spmd(
        nc, in_maps, core_ids=list(range(NCORES)))
    return assemble(res.results)



# revision 14
# speedup vs baseline: 1.0173x; 1.0113x over previous
"""Multi-head attention (B=2, L=2048, D=1024, H=16, DK=64) on 8 TRN2 NeuronCores.

Sharding: core c handles batch b = c//4 and head-group g = c%4 (4 heads,
256 model dims). Per-core compute (no collectives):
  QT/KT  [256, 2048] projections in [dk, seq] layout (rhs = x^T, lhsT = w^T)
  V      [2048, 256] with a fused ones-column per head (gives softmax Z free)
  S^T    = K_h @ Q_h^T per head in [keys, queries] layout (row-packed head
           pairs on the PE array, K=64 each)
  P      = exp(S^T / 8)     one wide ACT op per (key-tile, head-pair)
  ctx^T  = V'_h^T @ P       -> rows 0:64 ctx, row 64 = Z
  ctx   /= Z                (batched reciprocal + gpsimd partition broadcast)
  outT   = w_o-block-stationary matmul -> PARTIAL output [1024, 2048] bf16
Host sums the 4 head-group partials per batch, transposes, stacks batches.

Schedule: PE warm-up matmuls cover the initial DMA latency (HAM clock
ramp); the first real matmuls need only wk + x chunk 0, which are
prioritized on both HWDGE rings.  The attention rounds run 2 key tiles
per super-iteration (longer same-stationary matmul runs, fewer weight
-load exposures); KT chunks 1-3, all V tiles, remaining Q/K projections
and the output-projection units fill the interleave slots.  The last
query chunk's hp0 output half goes to a separate DRAM partial summed on
the host, so the tail is just division -> 8 matmuls -> stores.
"""

import numpy as np

D = 1024
L = 2048
DK = 64
GH = 4           # heads per core
GD = GH * DK     # model dims per core (256)
NCORES = 8

MM_DTYPE = "bfloat16"   # matmul operand dtype: bfloat16 | float32r | float32


def _build(mm_dtype: str = MM_DTYPE):
    import os
    import concourse.bacc as bacc
    import concourse.mybir as mybir
    import concourse.tile as tile

    del os

    f32 = mybir.dt.float32
    mmdt = getattr(mybir.dt, mm_dtype)
    Exp = mybir.ActivationFunctionType.Exp
    Copy = mybir.ActivationFunctionType.Copy

    ddt = mmdt if mm_dtype == "bfloat16" else f32
    odt = mmdt if mm_dtype == "bfloat16" else f32

    nc = bacc.Bacc("TRN2", target_bir_lowering=False, debug=False,
                   num_devices=NCORES)
    # Inputs are host-packed into the exact SBUF layouts (partition-major,
    # chunk-major for x) so every DMA line is 2-8KB contiguous -- ~3x the
    # descriptor-limited rate of the naive transposed layout.
    xh = nc.dram_tensor("xh", [128, 4, 8, 512], ddt, kind="ExternalInput").ap()
    wqh = nc.dram_tensor("wqh", [128, 8, GD], ddt, kind="ExternalInput").ap()
    wkh = nc.dram_tensor("wkh", [128, 8, GD], ddt, kind="ExternalInput").ap()
    wvh = nc.dram_tensor("wvh", [128, 8, GD], ddt, kind="ExternalInput").ap()
    woT = nc.dram_tensor("woT", [GD, D], ddt, kind="ExternalInput").ap()
    outT = nc.dram_tensor("outT", [D, L], odt, kind="ExternalOutput").ap()
    # hp0 half of the last query chunk's output partial (host adds it)
    outTb = nc.dram_tensor("outTb", [D, 512], odt, kind="ExternalOutput").ap()

    mdt = mmdt

    def dr(t):
        return t.bitcast(mmdt) if mm_dtype == "float32r" else t

    ND = D // 128    # 8 d-tiles
    NL = L // 128    # 16 key tiles
    NQ = L // 512    # 4 query chunks
    NE = D // 128    # 8 output-column blocks

    with tile.TileContext(nc) as tc:
        with (
            tc.tile_pool(name="xp", bufs=1) as xp,
            tc.tile_pool(name="wp", bufs=1) as wp,
            tc.tile_pool(name="qk", bufs=1) as qk,
            tc.tile_pool(name="vp", bufs=1) as vp,
            tc.tile_pool(name="cx", bufs=1) as cx,
            tc.tile_pool(name="pp", bufs=8) as pp,
            tc.tile_pool(name="rp", bufs=4) as rp,
            tc.tile_pool(name="op", bufs=3) as op_,
            tc.tile_pool(name="ps", bufs=2, space="PSUM") as ps,
            tc.tile_pool(name="pc", bufs=2, space="PSUM") as pc,
            tc.tile_pool(name="po", bufs=2, space="PSUM") as pop,
        ):
            # ---- PE warm-up: keep the HAM clock gate open while input
            # DMAs land.  Matmuls on a memset tile, results discarded.
            # Short: the first real projections are DMA-paced anyway.
            warm = wp.tile([128, 512], mdt, tag="warm", name="warm")
            nc.vector.memset(warm[:], 0.0)
            for _ in range(6):
                acc = pop.tile([128, 512], f32, tag="o", name="o")
                nc.tensor.matmul(acc[:], warm[:, 0:128], warm[:],
                                 start=True, stop=True)

            # ---- DMA inputs -------------------------------------------------
            # Critical path first on both HWDGE rings, split per d-tile so
            # the first projection's matmuls fire as each (wk_d, x_d) pair
            # lands instead of waiting for the whole block: sync ring carries
            # d 0-3, scalar ring d 4-7, pair-interleaved.
            wqts = wp.tile([128, ND, GD], mdt, tag="wq", name="wq")
            wkts = wp.tile([128, ND, GD], mdt, tag="wk", name="wk")
            wvts = wp.tile([128, ND, GD], mdt, tag="wv", name="wv")
            wots = [wp.tile([128, D], mdt, tag=f"wo{i}", name=f"wo{i}")
                    for i in range(2)]
            xta = xp.tile([128, ND, L], mdt, tag="x", name="x")
            xts = [xta[:, d, :] for d in range(ND)]

            # DMA engines round-robin between queues at packet granularity,
            # so bandwidth share is proportional to descriptor (line) size.
            # Keep critical lines fat: wk/x in 2-d-tile groups (1KB/2KB
            # lines), wq/wv as single whole-tensor transfers (4KB lines) on
            # the gpsimd queue ahead of wo.
            for g in range(2):
                ds = slice(2 * g, 2 * g + 2)
                nc.sync.dma_start(wkts[:, ds, :], dr(wkh[:, ds, :]))
                nc.sync.dma_start(xta[:, ds, 0:512], dr(xh[:, 0, ds, :]))
            for g in range(2, 4):
                ds = slice(2 * g, 2 * g + 2)
                nc.scalar.dma_start(wkts[:, ds, :], dr(wkh[:, ds, :]))
                nc.scalar.dma_start(xta[:, ds, 0:512], dr(xh[:, 0, ds, :]))
            nc.gpsimd.dma_start(wqts[:], dr(wqh[:]))
            nc.gpsimd.dma_start(wvts[:], dr(wvh[:]))
            for qtr in range(1, 4):
                sl = slice(qtr * 512, (qtr + 1) * 512)
                nc.sync.dma_start(xta[:, 0:4, sl], dr(xh[:, qtr, 0:4, :]))
                nc.scalar.dma_start(xta[:, 4:8, sl], dr(xh[:, qtr, 4:8, :]))
            for i in range(2):
                nc.gpsimd.dma_start(wots[i][:], dr(woT[i * 128:(i + 1) * 128, :]))

            # All-ones row for the final division: a K=1 matmul with this
            # stationary replicates a [1, 512] row across 64 PSUM partitions
            # (PE-based partition broadcast of 1/Z).
            bc_pat = wp.tile([1, 64], f32, tag="bcp", name="bcp")
            nc.vector.memset(bc_pat[:], 1.0)

            # ---- projection helpers (emitted just-in-time) ----------------
            vpa = vp.tile([128, NL * GH, DK + 1], mdt, tag="v", name="v")
            nc.vector.memset(vpa[:, :, DK:DK + 1], 1.0)

            qth = [qk.tile([128, L], mdt, tag=f"q{hp}", name=f"q{hp}")
                   for hp in range(2)]
            kth = [qk.tile([128, L], mdt, tag=f"k{hp}", name=f"k{hp}")
                   for hp in range(2)]
            ctxt = [cx.tile([128, L], mdt, tag=f"c{hp}", name=f"c{hp}")
                    for hp in range(2)]

            def proj_qk_chunk(hp, w_sb, dstl, qc, dorder=None):
                acc = pop.tile([128, 512], f32, tag="o", name="o")
                order = list(dorder) if dorder is not None else list(range(ND))
                for i, d in enumerate(order):
                    nc.tensor.matmul(
                        acc[:], w_sb[:, d, hp * 128:(hp + 1) * 128],
                        xts[d][:, qc * 512:(qc + 1) * 512],
                        start=(i == 0), stop=(i == ND - 1))
                nc.vector.tensor_copy(
                    dstl[hp][:, qc * 512:(qc + 1) * 512], acc[:])

            def v_proj_tile(lt):
                acc = pop.tile([128, 512], f32, tag="o", name="o")
                for d in range(ND):
                    nc.tensor.matmul(
                        acc[:, 0:GD],
                        xts[d][:, lt * 128:(lt + 1) * 128],
                        wvts[:, d, :],
                        start=(d == 0), stop=(d == ND - 1))
                nc.vector.tensor_copy(
                    vpa[:, lt * GH:(lt + 1) * GH, 0:DK],
                    acc[:, 0:GD].rearrange("p (h c) -> p h c", h=GH))

            def out_unit(eb, qc):
                esl = slice(eb * 128, (eb + 1) * 128)
                qsl = slice(qc * 512, (qc + 1) * 512)
                po = pop.tile([128, 512], f32, tag="o", name="o")
                for hp in range(2):
                    nc.tensor.matmul(po[:], wots[hp][:, esl],
                                     ctxt[hp][:, qsl],
                                     start=(hp == 0), stop=(hp == 1))
                ot = op_.tile([128, 512], mdt, tag="ot", name="ot")
                nc.vector.tensor_copy(ot[:], po[:])
                nc.sync.dma_start(outT[esl, qsl], ot[:])

            def oas_unit(eb):
                # hp0 half of the last query chunk -> separate DRAM partial
                qsl = slice((NQ - 1) * 512, NQ * 512)
                esl = slice(eb * 128, (eb + 1) * 128)
                po = pop.tile([128, 512], f32, tag="o", name="o")
                nc.tensor.matmul(po[:], wots[0][:, esl], ctxt[0][:, qsl],
                                 start=True, stop=True)
                ot = op_.tile([128, 512], mdt, tag="ot", name="ot")
                nc.vector.tensor_copy(ot[:], po[:])
                nc.scalar.dma_start(outTb[esl, :], ot[:])

            def attn(hp, qc, interleave=None, fast_div=False):
                """Attention for head-pair hp, query chunk qc.

                Two key tiles per super-iteration: scores for both tiles
                back-to-back (4 matmuls), then both exps, then both ctx
                accumulations (4 matmuls) -- longer same-kind runs hide
                more weight loads.  interleave: dict lt2 -> callable run
                between the scores and the ctx block of that super-iter.
                """
                qsl = slice(qc * 512, (qc + 1) * 512)
                cps = [pc.tile([DK + 1, 512], f32, tag="c", name=f"c{i}")
                       for i in range(2)]
                for lt2 in range(0, NL, 2):
                    sps = []
                    for lt in (lt2, lt2 + 1):
                        lsl = slice(lt * 128, (lt + 1) * 128)
                        sp = ps.tile([128, 1024], f32, tag="s", name="s")
                        sps.append(sp)
                        for i in range(2):
                            row = slice(i * 64, (i + 1) * 64)
                            nc.tensor.matmul(
                                sp[:, i * 512:(i + 1) * 512],
                                kth[hp][row, lsl], qth[hp][row, qsl],
                                start=True, stop=True,
                                tile_position=(i * 64, 0))
                    if interleave is not None and lt2 in interleave:
                        interleave[lt2]()
                    pts = []
                    for j, lt in enumerate((lt2, lt2 + 1)):
                        p = pp.tile([128, 1024], mdt, tag="p", name="p")
                        pts.append(p)
                        nc.scalar.activation(p[:], sps[j][:], Exp, scale=0.125)
                    for j, lt in enumerate((lt2, lt2 + 1)):
                        for i in range(2):
                            nc.tensor.matmul(
                                cps[i][:], vpa[:, lt * GH + 2 * hp + i, :],
                                pts[j][:, i * 512:(i + 1) * 512],
                                start=(lt == 0), stop=(lt == NL - 1))
                # divide by Z: evacuate both heads' psum, then reciprocals,
                # then broadcasts+muls.  fast_div orders for latency (the
                # reciprocals read the Z rows straight from PSUM so the
                # gpsimd broadcasts launch ASAP, overlapping the ctx
                # evacuation casts) -- used for the final round, whose
                # division gates the output tail.
                if fast_div:
                    # latency order: reciprocals read the Z rows straight
                    # from PSUM; 1/Z is partition-broadcast by a tiny PE
                    # matmul (also keeps the PE clock warm for the tail);
                    # the raw-ctx casts run on the scalar engine in
                    # parallel; one wide multiply normalizes both heads.
                    # keep the HAM clock gate open while the Z copies and
                    # reciprocal run (PE would otherwise idle and the tail
                    # matmuls would run at half clock)
                    for _ in range(4):
                        acc = pop.tile([128, 512], f32, tag="o", name="o")
                        nc.tensor.matmul(acc[:], warm[:, 0:128], warm[:],
                                         start=True, stop=True)
                    zz = rp.tile([1, 1024], f32, tag="zz2", name="zz2")
                    nc.vector.tensor_copy(zz[0:1, 0:512],
                                          cps[0][DK:DK + 1, :])
                    nc.scalar.activation(zz[0:1, 512:1024],
                                         cps[1][DK:DK + 1, :], Copy)
                    rz = rp.tile([1, 1024], f32, tag="rz2", name="rz2")
                    nc.vector.reciprocal_approx_fast(rz[:], zz[:])
                    cu = pp.tile([128, 512], mdt, tag="cuf", name="cuf")
                    for i in range(2):
                        nc.scalar.activation(cu[i * 64:(i + 1) * 64, :],
                                             cps[i][0:DK, :], Copy)
                    rzb = pop.tile([128, 512], f32, tag="o", name="o")
                    for i in range(2):
                        nc.tensor.matmul(rzb[i * 64:(i + 1) * 64, :],
                                         bc_pat[:],
                                         rz[0:1, i * 512:(i + 1) * 512],
                                         start=True, stop=True)
                    nc.vector.tensor_mul(ctxt[hp][:, qsl], cu[:], rzb[:])
                    return
                cus, rzs = [], []
                for i in range(2):
                    cu = pp.tile([64, 512], mdt, tag=f"cu{2*hp+i}",
                                 name=f"cu{2*hp+i}", bufs=2)
                    nc.vector.tensor_copy(cu[:], cps[i][0:DK, :])
                    zi = rp.tile([1, 512], f32, tag="zi", name="zi")
                    nc.vector.tensor_copy(zi[:], cps[i][DK:DK + 1, :])
                    cus.append(cu)
                    rzs.append(zi)
                for i in range(2):
                    rz = rp.tile([1, 512], f32, tag="rz", name="rz")
                    nc.vector.reciprocal_approx_fast(rz[:], rzs[i][:])
                    rzs[i] = rz
                for i in range(2):
                    rzb = rp.tile([64, 512], f32, tag="rzb", name="rzb")
                    nc.gpsimd.partition_broadcast(rzb[:], rzs[i][:])
                    nc.vector.tensor_mul(
                        ctxt[hp][i * 64:(i + 1) * 64, qsl],
                        cus[i][:], rzb[:])

            # ---- schedule ---------------------------------------------------
            # First projections consume d-tiles in DMA landing order
            # (sync ring delivers d 0-3 while scalar delivers d 4-7).
            DORD = [0, 1, 4, 5, 2, 3, 6, 7]
            proj_qk_chunk(0, wkts, kth, 0, dorder=DORD)
            proj_qk_chunk(0, wqts, qth, 0)

            def u(fn, *a):
                return lambda: fn(*a)

            def merge(*fns):
                def f():
                    for g in fns:
                        g()
                return f

            def vpp(a, b, *extra):
                return merge(u(v_proj_tile, a), u(v_proj_tile, b), *extra)

            # Round (0,0): all V tiles (needed by its own ctx), KT0 chunk
            # c needed by the scores of super-iter 4c, KT1 + QT1 c0 for
            # the next round.  High PE density is essential: the chip
            # throttles clocks when the PE stream goes sparse.
            attn(0, 0, interleave={
                0: vpp(0, 1),
                2: vpp(2, 3, u(proj_qk_chunk, 0, wkts, kth, 1)),
                4: vpp(4, 5, u(proj_qk_chunk, 1, wkts, kth, 0)),
                6: vpp(6, 7, u(proj_qk_chunk, 0, wkts, kth, 2)),
                8: vpp(8, 9, u(proj_qk_chunk, 1, wqts, qth, 0)),
                10: vpp(10, 11, u(proj_qk_chunk, 0, wkts, kth, 3)),
                12: vpp(12, 13, u(proj_qk_chunk, 1, wkts, kth, 1)),
                14: vpp(14, 15),
            })
            attn(1, 0, interleave={
                0: u(proj_qk_chunk, 1, wkts, kth, 2),
                4: u(proj_qk_chunk, 1, wkts, kth, 3),
                6: u(proj_qk_chunk, 0, wqts, qth, 1),
            })
            # out units lag their query chunk by one round and start at
            # slot 4 (the producing division trails the previous round).
            attn(0, 1, interleave={
                0: u(proj_qk_chunk, 1, wqts, qth, 1),
                4: u(out_unit, 0, 0), 8: u(out_unit, 1, 0),
                12: u(out_unit, 2, 0), 14: u(out_unit, 3, 0),
            })
            attn(1, 1, interleave={
                0: u(proj_qk_chunk, 0, wqts, qth, 2),
                4: u(out_unit, 4, 0), 8: u(out_unit, 5, 0),
                12: u(out_unit, 6, 0), 14: u(out_unit, 7, 0),
            })
            attn(0, 2, interleave={
                0: u(proj_qk_chunk, 1, wqts, qth, 2),
                4: u(out_unit, 0, 1), 8: u(out_unit, 1, 1),
                12: u(out_unit, 2, 1), 14: u(out_unit, 3, 1),
            })
            attn(1, 2, interleave={
                0: u(proj_qk_chunk, 0, wqts, qth, 3),
                4: u(out_unit, 4, 1), 8: u(out_unit, 5, 1),
                12: u(out_unit, 6, 1), 14: u(out_unit, 7, 1),
            })
            attn(0, 3, interleave={
                0: u(proj_qk_chunk, 1, wqts, qth, 3),
                4: u(out_unit, 0, 2), 8: u(out_unit, 1, 2),
                12: u(out_unit, 2, 2), 14: u(out_unit, 3, 2),
            })
            attn(1, 3, interleave={
                2: merge(u(out_unit, 4, 2), u(out_unit, 5, 2)),
                4: merge(u(out_unit, 6, 2), u(out_unit, 7, 2)),
                6: merge(u(oas_unit, 0), u(oas_unit, 1)),
                8: merge(u(oas_unit, 2), u(oas_unit, 3)),
                10: merge(u(oas_unit, 4), u(oas_unit, 5)),
                12: merge(u(oas_unit, 6), u(oas_unit, 7)),
            }, fast_div=True)
            # Tail: hp1 halves of the last chunk.  Evacuations alternate
            # between the vector and scalar engines and the stores between
            # both HWDGE rings so the drain pipelines.
            qsl = slice((NQ - 1) * 512, NQ * 512)
            for eb in range(NE):
                esl = slice(eb * 128, (eb + 1) * 128)
                po = pop.tile([128, 512], f32, tag="o", name="o")
                nc.tensor.matmul(po[:], wots[1][:, esl], ctxt[1][:, qsl],
                                 start=True, stop=True)
                ot = op_.tile([128, 512], mdt, tag="ot", name="ot")
                if eb % 2 == 0:
                    nc.vector.tensor_copy(ot[:], po[:])
                    nc.sync.dma_start(outT[esl, qsl], ot[:])
                else:
                    nc.scalar.activation(ot[:], po[:], Copy)
                    nc.scalar.dma_start(outT[esl, qsl], ot[:])
    nc.compile()
    return nc


_CACHED = {}


def _get_nc(mm_dtype: str = MM_DTYPE):
    if mm_dtype not in _CACHED:
        _CACHED[mm_dtype] = _build(mm_dtype)
    return _CACHED[mm_dtype]


def _round_fp32r(a):
    """Round-to-nearest-even fp32 -> fp32r (11 explicit mantissa bits)."""
    u = np.ascontiguousarray(a, np.float32).view(np.uint32).copy()
    u += 0x7FF + ((u >> 12) & 1)
    u &= 0xFFFFF000
    return u.view(np.float32)


def make_in_maps(x, w_qkv, w_o):
    if MM_DTYPE == "float32r":
        cvt = _round_fp32r
    elif MM_DTYPE == "bfloat16":
        import ml_dtypes
        cvt = lambda a: np.asarray(a, dtype=ml_dtypes.bfloat16)  # noqa: E731
    else:
        cvt = lambda a: a  # noqa: E731
    wq, wk, wv = (w_qkv[i * D:(i + 1) * D] for i in range(3))

    def pack_x(xb):
        # [L, D] -> [128, NQ, ND, 512]: xh[p, qc, d, c] = xb[qc*512+c, d*128+p]
        return np.ascontiguousarray(
            xb.reshape(4, 512, 8, 128).transpose(3, 0, 2, 1))

    def pack_w(wg):
        # [GD, D] -> [128, ND, GD]: wh[p, d, c] = wg.T[d*128+p, c]
        return np.ascontiguousarray(
            wg.T.reshape(8, 128, GD).transpose(1, 0, 2))

    in_maps = []
    for c in range(NCORES):
        b, g = divmod(c, 4)
        gs = slice(g * GD, (g + 1) * GD)
        in_maps.append({
            "xh": cvt(pack_x(x[b])),
            "wqh": cvt(pack_w(wq[gs])),
            "wkh": cvt(pack_w(wk[gs])),
            "wvh": cvt(pack_w(wv[gs])),
            "woT": cvt(np.ascontiguousarray(w_o[:, gs].T)),
        })
    return in_maps


def assemble(results):
    out = np.empty((2, L, D), np.float32)
    for b in range(2):
        acc = np.asarray(results[4 * b]["outT"], np.float32)
        accb = np.asarray(results[4 * b]["outTb"], np.float32)
        for g in range(1, 4):
            acc = acc + np.asarray(results[4 * b + g]["outT"], np.float32)
            accb = accb + np.asarray(results[4 * b + g]["outTb"], np.float32)
        acc[:, (L - 512):] += accb
        out[b] = acc.T
    return out


def kernel(x, w_qkv, w_o):
    from concourse import bass_utils
    nc = _get_nc()
    in_maps = make_in_maps(np.asarray(x, np.float32),
                           np.asarray(w_qkv, np.float32),
                           np.asarray(w_o, np.float32))
    res = bass_utils.run_bass_kernel_spmd(
        nc, in_maps, core_ids=list(range(NCORES)))
    return assemble(res.results)

